# revision 30
# baseline (speedup 1.0000x reference)
"""MobileAttention3D Trainium2 kernel (8-core SPMD), v2.

Sharding: core c -> (b = c//4, hg = c%4) owns batch b and H rows
[8*hg, 8*hg+8).  All conv GEMMs + attention for that slice run locally;
the only cross-core communication is a 16KB-per-half AllReduce of
partial attention logits within each batch group {0..3}, {4..7}.

v2 changes vs v1 (trace-driven):
- All PSUM drains write CONTIGUOUS SBUF runs; the layout scrambles
  moved into strided stationary/moving matmul APs (free on PE).
  v1's q-conv drains were 4.7 ns/elem scattered writes; now ~0.9.
- Attention logits are split by dq-half (mu), so AllReduce mu0 issues
  mid-q-conv and its ~40us latency hides under remaining convs; av+proj
  are also split per mu so AR1 hides under av0/proj0.
- v-transpose batches 4 vd values per PE transpose (128-partition
  output instead of 32), cutting vT PE time 4x, and emits the vd-quad
  layout that row-tiled av needs.
- av runs on 4 concurrent 32x128 PE row-tiles (contract dim dk=32),
  via tile_position auto-inference from AP base partitions.
- proj weights are row-permuted on the host so av outputs land
  PSUM-partition-aligned with the proj contraction.

Token order per core: t = d*256 + eta*128 + hw, hw = (h_loc%4)*32 + w,
eta = h_loc//4.  Attention feature f = hw_full*64 + c factors through
the torch reshape as (w' , c2): w' = h_loc*4 + w//8 (output W index),
c2 = (w%8)*64 + c (proj input channel).  Head n becomes output H:
h' = n*4 + hg.
"""

import numpy as np
import ml_dtypes

NH, KD, VD, C = 8, 64, 64, 256
B, D, H, W = 2, 32, 32, 32
HS = H // 4            # h rows per core
T = D * HS * W         # 8192 tokens per core
P = 128
NCORES = 8
SCALE = float(VD) ** -0.5

_CACHE = {}


def _build(has_qb, has_kvb, has_pb, sim_mode=False):
    import concourse.bacc as bacc
    import concourse.mybir as mybir
    from concourse import tile

    dt = mybir.dt
    f32, bf16 = dt.float32, dt.bfloat16
    AX = mybir.AxisListType
    AF = mybir.ActivationFunctionType

    nc = bacc.Bacc("TRN2", target_bir_lowering=False, debug=False,
                   enable_asserts=False,
                   num_devices=1 if sim_mode else NCORES)

    x_in = nc.dram_tensor("x", [C, T], bf16, kind="ExternalInput")
    wq_in = nc.dram_tensor("wq", [C, NH * KD], bf16, kind="ExternalInput")
    wkv_in = nc.dram_tensor("wkv", [C, KD + VD], bf16, kind="ExternalInput")
    # proj weights, row-permuted: wps[j2, vi, c] = (proj_w.T*ls)[w8*64+16*vi+v0, c]
    wp_in = nc.dram_tensor("wp", [P, 4 * C], bf16, kind="ExternalInput")
    idt_in = nc.dram_tensor("idt", [P, P], bf16, kind="ExternalInput")
    qb_in = kvb_in = pb_in = None
    if has_qb:
        qb_in = nc.dram_tensor("qb", [P, NH * KD], bf16, kind="ExternalInput")
    if has_kvb:
        kvb_in = nc.dram_tensor("kvb", [P, KD + VD], bf16, kind="ExternalInput")
    if has_pb:
        # proj bias pre-multiplied by layer_scale, per C channel
        pb_in = nc.dram_tensor("pb", [C, 1], f32, kind="ExternalInput")
    out_t = nc.dram_tensor("out", [C, T], f32, kind="ExternalOutput")

    with tile.TileContext(nc) as tc:
        with tc.tile_pool(name="wpool", bufs=1) as wpool, \
             tc.tile_pool(name="xp", bufs=1) as xpool, \
             tc.tile_pool(name="big", bufs=1) as bigpool, \
             tc.tile_pool(name="kvp", bufs=1) as kvpool, \
             tc.tile_pool(name="small", bufs=1) as spool, \
             tc.tile_pool(name="stage", bufs=4) as stpool, \
             tc.tile_pool(name="psum", bufs=4, space="PSUM") as psum, \
             tc.tile_pool(name="dram", bufs=1, space="DRAM") as dram:

            wq = wpool.tile([P, 2, NH * KD], bf16)
            wkv = wpool.tile([P, 2, KD + VD], bf16)
            wps = wpool.tile([P, 4, C], bf16)
            idt = wpool.tile([P, P], bf16)
            x_sb = xpool.tile([P, 2, T], bf16)

            # weights first (tiny), then x chunks in d-order; all on SP queue
            for ci in range(2):
                nc.sync.dma_start(wkv[:, ci, :], wkv_in[ci * P:(ci + 1) * P, :])
                nc.sync.dma_start(wq[:, ci, :], wq_in[ci * P:(ci + 1) * P, :])
            XCH = 8
            for g in range(XCH):
                lo, hi = g * (T // XCH), (g + 1) * (T // XCH)
                if g == 0:
                    # halved first transfers so the kv conv starts sooner
                    mid = (lo + hi) // 2
                    for ci in range(2):
                        nc.sync.dma_start(x_sb[:, ci, lo:mid],
                                          x_in[ci * P:(ci + 1) * P, lo:mid])
                    for ci in range(2):
                        nc.sync.dma_start(x_sb[:, ci, mid:hi],
                                          x_in[ci * P:(ci + 1) * P, mid:hi])
                    nc.sync.dma_start(idt[:], idt_in[:])
                    nc.sync.dma_start(wps[:], wp_in[:])
                else:
                    for ci in range(2):
                        nc.sync.dma_start(x_sb[:, ci, lo:hi],
                                          x_in[ci * P:(ci + 1) * P, lo:hi])
            qb = kvb = pb = None
            if has_qb:
                qb = wpool.tile([P, NH * KD], bf16)
                nc.sync.dma_start(qb[:], qb_in[:])
            if has_kvb:
                kvb = wpool.tile([P, KD + VD], bf16)
                nc.sync.dma_start(kvb[:], kvb_in[:])
            if has_pb:
                pb = wpool.tile([P, 2, 1], f32)
                for ci in range(2):
                    nc.sync.dma_start(pb[:, ci, :], pb_in[ci * P:(ci + 1) * P, :])

            # Q2[p=hw, d, eta, (n,kd)] : q-conv output, drain-contiguous
            Q2 = bigpool.tile([P, D * 2 * 512], bf16, tag="big", name="Q2")
            # kvsb[p=hw, d, eta, ch(kd|vd)]
            kvsb = kvpool.tile([P, D * 2 * (KD + VD)], bf16)
            # vst[p=hw, eta, v0, vi, dk]: Pool-engine staging so the v
            # transpose input is one contiguous 128-run per (eta, v0)
            vst = kvpool.tile([P, 2 * 16 * 4 * D], bf16)
            # vq[p = vi*32+dk, eta, j2=(v0,w8), x=(hl,wh)] : quad v^T
            vq = kvpool.tile([P, 2 * 16 * P], bf16)

            attn4 = spool.tile([P, 2, P], bf16)    # [(dqh,n), mu, (vi,dk)]
            attnT4 = spool.tile([P, 2, P], bf16)   # [(vi,dk), mu, (dqh,n)]
            l2 = spool.tile([P, 2, 32], f32)
            l3 = spool.tile([P, 2, 32], f32)
            ex = spool.tile([P, 2, 32], f32)
            red = spool.tile([P, 8], f32)

            arin = [dram.tile([P, 32], f32, name=f"arin{mu}")
                    for mu in range(2)]
            arout = [dram.tile([P, 32], f32, name=f"arout{mu}")
                     for mu in range(2)]

            # ---- kv conv groups: tokens on partitions; drain contiguous ----
            Kv = kvsb.rearrange("p (d e c) -> p e c d", d=D, e=2, c=KD + VD)

            def kvhalf(mp, h):
                ps = psum.tile([P, 512], f32, tag="ps", name=f"pskv{mp}_{h}")
                for jj in range(4):
                    j = 8 * mp + 4 * h + jj
                    for ci in range(2):
                        nc.tensor.matmul(
                            ps[:, jj * P:(jj + 1) * P],
                            x_sb[:, ci, j * P:(j + 1) * P],
                            wkv[:, ci, :],
                            start=(ci == 0), stop=(ci == 1))
                dst = kvsb[:, mp * 1024 + h * 512:mp * 1024 + (h + 1) * 512]
                if has_kvb:
                    nc.any.tensor_tensor(
                        dst.rearrange("p (t c) -> p t c", c=P),
                        ps.rearrange("p (t c) -> p t c", c=P),
                        kvb.rearrange("p c -> p 1 c")[:, [0] * 4, :],
                        op=mybir.AluOpType.add)
                else:
                    nc.any.tensor_copy(dst, ps[:])

            def kvgroup(mp):
                kvhalf(mp, 0)
                kvhalf(mp, 1)

            # ---- q pass (d-ordered) + qk/AR per dq-half (mu) ----
            # Q2[p=hw, eta, d, c=(n,kd)]: drain lands in 512-contiguous runs
            # and the qk stationary [(dqh,n)-slice] is a stride-64 run.
            Q2d = Q2.rearrange("p (e d c) -> p e d c", e=2, d=D)
            # qk stationary view: [p, eta, mu, j=(dqh*8+n), kd]
            Q2v = Q2.rearrange("p (e m j k) -> p e m j k", e=2, m=2, k=KD)

            def qconv(d):
                ps = psum.tile([P, 1024], f32, tag="ps", name=f"psq{d}")
                for eta in range(2):
                    j = d * 2 + eta
                    for ci in range(2):
                        nc.tensor.matmul(ps[:, eta * 512:(eta + 1) * 512],
                                         x_sb[:, ci, j * P:(j + 1) * P],
                                         wq[:, ci, :],
                                         start=(ci == 0), stop=(ci == 1))
                if has_qb:
                    nc.any.tensor_tensor(
                        Q2d[:, :, d, :],
                        ps.rearrange("p (e f) -> p e f", e=2),
                        qb.rearrange("p f -> p 1 f")[:, [0, 0], :],
                        op=mybir.AluOpType.add)
                else:
                    nc.any.tensor_copy(Q2d[:, :, d, :],
                                       ps.rearrange("p (e f) -> p e f", e=2))

            def qk_ar(mu):
                psL = psum.tile([P, 32], f32, tag="ps", name=f"psL{mu}")
                for idx, (eta, kd) in enumerate((e, k) for e in range(2)
                                                for k in range(KD)):
                    nc.tensor.matmul(
                        psL[:],
                        Q2v[:, eta, mu, :, kd],
                        Kv[:, eta, kd, :],
                        start=(idx == 0), stop=(idx == 127))
                nc.any.tensor_copy(l2[:, mu, :], psL[:])
                nc.sync.dma_start(arin[mu][:], l2[:, mu, :])
                if sim_mode:
                    nc.sync.dma_start(arout[mu][:], arin[mu][:])
                else:
                    nc.gpsimd.collective_compute(
                        "AllReduce", mybir.AluOpType.add,
                        replica_groups=[[0, 1, 2, 3], [4, 5, 6, 7]],
                        ins=[arin[mu].opt()], outs=[arout[mu].opt()])

            # kv groups interleaved with d<16 q-convs (chunk g feeds both);
            # kv must fully finish before qk0, so d>=16 q-convs come after.
            for g in range(8):
                kvgroup(g)
                if g < 4:
                    for d in range(4 * g, 4 * g + 4):
                        qconv(d)
            # v re-layout on the otherwise-idle Pool engine (runs under qk/AR)
            Vsrc = kvsb.rearrange("p (d e ck vi v0) -> p ck e v0 vi d",
                                  d=D, e=2, ck=2, vi=4, v0=16)
            nc.gpsimd.tensor_copy(
                vst.rearrange("p (e v0 vi d) -> p e v0 vi d",
                              e=2, v0=16, vi=4),
                Vsrc[:, 1, :, :, :, :])
            qk_ar(0)
            for d in range(16, 32):
                qconv(d)
            qk_ar(1)
            # l3 loads emitted after both arin DMAs: the SP queue is
            # in-order, and l3[0] blocks on AR0 completion.
            for mu in range(2):
                nc.sync.dma_start(l3[:, mu, :], arout[mu][:])

            # ---- v transpose -> vq quads [vi*32+dk, (eta, v0, hw')] ----
            # hw' = w8*16 + hl*4 + wh (host-permuted token order), so the
            # av stationary col j2 = v0*8 + w8 is a stride-16 run.
            vstv = vst.rearrange("p (e v0 f) -> p e v0 f", e=2, v0=16)
            vqv = vq.rearrange("p (e v0 hw) -> p e v0 hw", e=2, v0=16)
            for eta in range(2):
                for g8 in range(2):
                    ps = psum.tile([P, 1024], bf16, tag="ps",
                                   name=f"psv{eta}_{g8}")
                    for l in range(8):
                        v0 = g8 * 8 + l
                        nc.tensor.transpose(ps[:, l * P:(l + 1) * P],
                                            vstv[:, eta, v0, :], idt[:])
                    nc.any.tensor_copy(
                        vqv[:, eta, g8 * 8:(g8 + 1) * 8, :],
                        ps.rearrange("p (l hw) -> p l hw", l=8))

            # oo[p=j2(v0,w8), c=vi, mu, w', nd] reuses Q2's slot
            oo = bigpool.tile([P, 4 * 2 * 32 * P], bf16, tag="big", name="oo")
            Oov = oo.rearrange("p (c m w nd) -> p c m w nd",
                               c=4, m=2, w=32)
            # av stationary view: [p=(vi,dk), eta, x=(hl,wh), j2]  (stride 16)
            Vat = vq.rearrange("p (e j2 x) -> p e x j2", e=2, j2=P, x=16)

            def softmax(mu):
                sl = l3[:, mu, :]
                mx = red[:, mu * 4 + 0: mu * 4 + 1]
                mxn = red[:, mu * 4 + 1: mu * 4 + 2]
                sm = red[:, mu * 4 + 2: mu * 4 + 3]
                rs = red[:, mu * 4 + 3: mu * 4 + 4]
                nc.vector.reduce_max(mx, sl, axis=AX.X, op=mybir.AluOpType.max)
                nc.scalar.mul(mxn, mx, -SCALE)
                nc.scalar.activation(ex[:, mu, :], sl, AF.Exp,
                                     bias=mxn, scale=SCALE, accum_out=sm)
                nc.vector.reciprocal(rs, sm)
                for vi in range(4):
                    nc.any.tensor_scalar_mul(
                        attn4[:, mu, vi * 32:(vi + 1) * 32], ex[:, mu, :], rs)

            def attn_t(mu):
                # attn transpose -> attnT4 [(vi,dk), (dqh,n)]
                pst = psum.tile([P, P], bf16, tag="ps", name=f"psat{mu}")
                nc.tensor.transpose(pst[:], attn4[:, mu, :], idt[:])
                nc.any.tensor_copy(attnT4[:, mu, :], pst[:])

            def av(mu):
                # av on 4 PE row-tiles (32x128), tile vi = vd quad
                for gg in range(4):      # 8 w' per psum tile
                    pls = [psum.tile([P, 1024], f32, tag="ps",
                                     name=f"psav{mu}_{gg}_{vi}")
                           for vi in range(4)]
                    for wl in range(8):
                        wp_ = gg * 8 + wl
                        e_w, x_w = wp_ // 16, wp_ % 16
                        for vi in range(4):
                            nc.tensor.matmul(
                                pls[vi][:, wl * P:(wl + 1) * P],
                                Vat[vi * 32:(vi + 1) * 32, e_w, x_w, :],
                                attnT4[vi * 32:(vi + 1) * 32, mu, :],
                                start=True, stop=True,
                                tile_position=(vi * 32, 0))
                    for vi in range(4):
                        nc.any.tensor_copy(
                            Oov[:, vi, mu, gg * 8:(gg + 1) * 8, :],
                            pls[vi].rearrange("p (w nd) -> p w nd", w=8))

            outv = out_t.rearrange("(ct p) t -> p ct t", p=P)

            def proj(mu):
                # proj (this mu's tokens) + layer_scale -> out
                for gg in range(4):      # 1024 tokens per gg
                    for ct in range(2):
                        ps = psum.tile([P, 1024], f32, tag="ps",
                                       name=f"psp{mu}_{gg}_{ct}")
                        for half in range(2):
                            for vi in range(4):
                                nc.tensor.matmul(
                                    ps[:, half * 512:(half + 1) * 512],
                                    wps[:, vi, ct * P:(ct + 1) * P],
                                    oo[:, vi * 8192 + mu * 4096 + gg * 1024
                                       + half * 512:
                                       vi * 8192 + mu * 4096 + gg * 1024
                                       + half * 512 + 512],
                                    start=(vi == 0), stop=(vi == 3))
                        stg = stpool.tile([P, 1024], f32, tag="stg",
                                          name=f"stg{mu}_{gg}_{ct}")
                        odd = (gg * 2 + ct) % 2
                        if has_pb:
                            nc.any.tensor_scalar_add(stg[:], ps[:],
                                                     pb[:, ct, :])
                        elif odd:
                            nc.scalar.copy(stg[:], ps[:])
                        else:
                            nc.vector.tensor_copy(stg[:], ps[:])
                        nc.sync.dma_start(
                            outv[:, ct,
                                 mu * 4096 + gg * 1024:mu * 4096 + (gg + 1) * 1024],
                            stg[:])

            # mu1's softmax is emitted before proj0 so its Scalar/Vector ops
            # don't queue behind proj0's PSUM drains; its PE transpose comes
            # after proj0 so a late AR1 can't stall proj0's matmuls.
            softmax(0)
            attn_t(0)
            av(0)
            softmax(1)
            proj(0)
            attn_t(1)
            av(1)
            proj(1)

    nc.finalize()
    return nc


def _get_nc(has_qb, has_kvb, has_pb):
    key = (has_qb, has_kvb, has_pb)
    if key not in _CACHE:
        _CACHE[key] = _build(*key)
    return _CACHE[key]


def kernel(x, q_w, q_b, kv_w, kv_b, proj_w, proj_b, layer_scale):
    from concourse.bass_utils import run_bass_kernel_spmd
    import os

    x = np.asarray(x, dtype=np.float32)
    q_w = np.asarray(q_w, dtype=np.float32)
    q_b = np.asarray(q_b, dtype=np.float32)
    kv_w = np.asarray(kv_w, dtype=np.float32)
    kv_b = np.asarray(kv_b, dtype=np.float32)
    proj_w = np.asarray(proj_w, dtype=np.float32)
    proj_b = np.asarray(proj_b, dtype=np.float32)
    layer_scale = np.asarray(layer_scale, dtype=np.float32)

    has_qb = bool(np.any(q_b != 0))
    has_kvb = bool(np.any(kv_b != 0))
    has_pb = bool(np.any(proj_b != 0))
    nc = _get_nc(has_qb, has_kvb, has_pb)

    bf = ml_dtypes.bfloat16
    ls_c = layer_scale.reshape(C)                          # [C] f32
    wq = np.ascontiguousarray(q_w.T).astype(bf)            # [C, 512]
    wkv = np.ascontiguousarray(kv_w.T).astype(bf)          # [C, 128]
    # fold layer_scale into proj weights; row j2 = v0*8 + w8 of block vi
    # holds c2 = w8*64 + 16*vi + v0
    wp_full = (proj_w * ls_c[:, None]).T                   # [512 = c2, C]
    j2 = np.arange(P)
    rows = (j2 % 8)[:, None] * 64 + 16 * np.arange(4)[None, :] \
        + (j2 // 8)[:, None]                               # [128, 4vi]
    wps = np.ascontiguousarray(
        wp_full[rows].reshape(P, 4 * C)).astype(bf)        # [128, 4*C]
    idt = np.eye(P, dtype=bf)

    shared = {"wq": wq, "wkv": wkv, "wp": wps, "idt": idt}
    if has_qb:
        shared["qb"] = np.broadcast_to(q_b.astype(bf), (P, NH * KD)).copy()
    if has_kvb:
        shared["kvb"] = np.broadcast_to(kv_b.astype(bf), (P, KD + VD)).copy()
    if has_pb:
        shared["pb"] = (proj_b * layer_scale.reshape(-1)).reshape(C, 1) \
            .astype(np.float32)

    in_maps = []
    for c in range(NCORES):
        b, hg = c // 4, c % 4
        # token order: (d, eta=hl//4, w8=w%8, hl=h%4, wh=w//8)
        xs = x[b, :, :, hg * HS:(hg + 1) * HS, :] \
            .reshape(C, D, 2, 4, 4, 8).transpose(0, 1, 2, 5, 3, 4)
        xc = np.ascontiguousarray(xs.reshape(C, T)).astype(bf)
        in_maps.append({"x": xc, **shared})

    trace = bool(int(os.environ.get("KERNEL_TRACE", "0")))
    res = run_bass_kernel_spmd(nc, in_maps, core_ids=list(range(NCORES)),
                               trace=trace)
    kernel.last_results = res

    out = np.empty((B, C, D, H, W), dtype=np.float32)
    for c in range(NCORES):
        b, hg = c // 4, c % 4
        # res T enum = (mu, w', dqh, n): d = mu*16+dqh, h' = n*4+hg
        r = res.results[c]["out"].reshape(C, 2, 32, 16, NH)
        r = r.transpose(0, 1, 3, 4, 2).reshape(C, D, NH, 32)
        out[b, :, :, hg::4, :] = r
    return out


# revision 31
# speedup vs baseline: 1.0065x; 1.0065x over previous
"""MobileAttention3D Trainium2 kernel (8-core SPMD), v2.

Sharding: core c -> (b = c//4, hg = c%4) owns batch b and H rows
[8*hg, 8*hg+8).  All conv GEMMs + attention for that slice run locally;
the only cross-core communication is a 16KB-per-half AllReduce of
partial attention logits within each batch group {0..3}, {4..7}.

v2 changes vs v1 (trace-driven):
- All PSUM drains write CONTIGUOUS SBUF runs; the layout scrambles
  moved into strided stationary/moving matmul APs (free on PE).
  v1's q-conv drains were 4.7 ns/elem scattered writes; now ~0.9.
- Attention logits are split by dq-half (mu), so AllReduce mu0 issues
  mid-q-conv and its ~40us latency hides under remaining convs; av+proj
  are also split per mu so AR1 hides under av0/proj0.
- v-transpose batches 4 vd values per PE transpose (128-partition
  output instead of 32), cutting vT PE time 4x, and emits the vd-quad
  layout that row-tiled av needs.
- av runs on 4 concurrent 32x128 PE row-tiles (contract dim dk=32),
  via tile_position auto-inference from AP base partitions.
- proj weights are row-permuted on the host so av outputs land
  PSUM-partition-aligned with the proj contraction.

Token order per core: t = d*256 + eta*128 + hw, hw = (h_loc%4)*32 + w,
eta = h_loc//4.  Attention feature f = hw_full*64 + c factors through
the torch reshape as (w' , c2): w' = h_loc*4 + w//8 (output W index),
c2 = (w%8)*64 + c (proj input channel).  Head n becomes output H:
h' = n*4 + hg.
"""

import numpy as np
import ml_dtypes

NH, KD, VD, C = 8, 64, 64, 256
B, D, H, W = 2, 32, 32, 32
HS = H // 4            # h rows per core
T = D * HS * W         # 8192 tokens per core
P = 128
NCORES = 8
SCALE = float(VD) ** -0.5

_CACHE = {}


def _build(has_qb, has_kvb, has_pb, sim_mode=False):
    import concourse.bacc as bacc
    import concourse.mybir as mybir
    from concourse import tile

    dt = mybir.dt
    f32, bf16 = dt.float32, dt.bfloat16
    AX = mybir.AxisListType
    AF = mybir.ActivationFunctionType

    nc = bacc.Bacc("TRN2", target_bir_lowering=False, debug=False,
                   enable_asserts=False,
                   num_devices=1 if sim_mode else NCORES)

    x_in = nc.dram_tensor("x", [C, T], bf16, kind="ExternalInput")
    wq_in = nc.dram_tensor("wq", [C, NH * KD], bf16, kind="ExternalInput")
    wkv_in = nc.dram_tensor("wkv", [C, KD + VD], bf16, kind="ExternalInput")
    # proj weights, row-permuted: wps[j2, vi, c] = (proj_w.T*ls)[w8*64+16*vi+v0, c]
    wp_in = nc.dram_tensor("wp", [P, 4 * C], bf16, kind="ExternalInput")
    idt_in = nc.dram_tensor("idt", [P, P], bf16, kind="ExternalInput")
    qb_in = kvb_in = pb_in = None
    if has_qb:
        qb_in = nc.dram_tensor("qb", [P, NH * KD], bf16, kind="ExternalInput")
    if has_kvb:
        kvb_in = nc.dram_tensor("kvb", [P, KD + VD], bf16, kind="ExternalInput")
    if has_pb:
        # proj bias pre-multiplied by layer_scale, per C channel
        pb_in = nc.dram_tensor("pb", [C, 1], f32, kind="ExternalInput")
    out_t = nc.dram_tensor("out", [C, T], f32, kind="ExternalOutput")

    with tile.TileContext(nc) as tc:
        with tc.tile_pool(name="wpool", bufs=1) as wpool, \
             tc.tile_pool(name="xp", bufs=1) as xpool, \
             tc.tile_pool(name="big", bufs=1) as bigpool, \
             tc.tile_pool(name="kvp", bufs=1) as kvpool, \
             tc.tile_pool(name="small", bufs=1) as spool, \
             tc.tile_pool(name="stage", bufs=4) as stpool, \
             tc.tile_pool(name="psum", bufs=4, space="PSUM") as psum, \
             tc.tile_pool(name="dram", bufs=1, space="DRAM") as dram:

            wq = wpool.tile([P, 2, NH * KD], bf16)
            wkv = wpool.tile([P, 2, KD + VD], bf16)
            wps = wpool.tile([P, 4, C], bf16)
            idt = wpool.tile([P, P], bf16)
            x_sb = xpool.tile([P, 2, T], bf16)

            # weights first (tiny), then x chunks in d-order; all on SP queue
            for ci in range(2):
                nc.sync.dma_start(wkv[:, ci, :], wkv_in[ci * P:(ci + 1) * P, :])
                nc.sync.dma_start(wq[:, ci, :], wq_in[ci * P:(ci + 1) * P, :])
            XCH = 8
            for g in range(XCH):
                lo, hi = g * (T // XCH), (g + 1) * (T // XCH)
                for ci in range(2):
                    nc.sync.dma_start(x_sb[:, ci, lo:hi],
                                      x_in[ci * P:(ci + 1) * P, lo:hi])
                if g == 0:
                    nc.sync.dma_start(idt[:], idt_in[:])
                    nc.sync.dma_start(wps[:], wp_in[:])
            qb = kvb = pb = None
            if has_qb:
                qb = wpool.tile([P, NH * KD], bf16)
                nc.sync.dma_start(qb[:], qb_in[:])
            if has_kvb:
                kvb = wpool.tile([P, KD + VD], bf16)
                nc.sync.dma_start(kvb[:], kvb_in[:])
            if has_pb:
                pb = wpool.tile([P, 2, 1], f32)
                for ci in range(2):
                    nc.sync.dma_start(pb[:, ci, :], pb_in[ci * P:(ci + 1) * P, :])

            # Q2[p=hw, d, eta, (n,kd)] : q-conv output, drain-contiguous
            Q2 = bigpool.tile([P, D * 2 * 512], bf16, tag="big", name="Q2")
            # kvsb[p=hw, d, eta, ch(kd|vd)]
            kvsb = kvpool.tile([P, D * 2 * (KD + VD)], bf16)
            # vst[p=hw, eta, v0, vi, dk]: Pool-engine staging so the v
            # transpose input is one contiguous 128-run per (eta, v0)
            vst = kvpool.tile([P, 2 * 16 * 4 * D], bf16)
            # vq[p = vi*32+dk, eta, j2=(v0,w8), x=(hl,wh)] : quad v^T
            vq = kvpool.tile([P, 2 * 16 * P], bf16)

            attn4 = spool.tile([P, 2, P], bf16)    # [(dqh,n), mu, (vi,dk)]
            attnT4 = spool.tile([P, 2, P], bf16)   # [(vi,dk), mu, (dqh,n)]
            l2 = spool.tile([P, 2, 32], f32)
            l3 = spool.tile([P, 2, 32], f32)
            ex = spool.tile([P, 2, 32], f32)
            red = spool.tile([P, 8], f32)

            arin = [dram.tile([P, 32], f32, name=f"arin{mu}")
                    for mu in range(2)]
            arout = [dram.tile([P, 32], f32, name=f"arout{mu}")
                     for mu in range(2)]

            # ---- kv conv groups: tokens on partitions; drain contiguous ----
            Kv = kvsb.rearrange("p (d e c) -> p e c d", d=D, e=2, c=KD + VD)

            def kvgroup(mp):
                ps = psum.tile([P, 1024], f32, tag="ps", name=f"pskv{mp}")
                for jj in range(8):
                    j = 8 * mp + jj
                    for ci in range(2):
                        nc.tensor.matmul(
                            ps[:, jj * P:(jj + 1) * P],
                            x_sb[:, ci, j * P:(j + 1) * P],
                            wkv[:, ci, :],
                            start=(ci == 0), stop=(ci == 1))
                dst = kvsb[:, mp * 1024:(mp + 1) * 1024]
                if has_kvb:
                    nc.any.tensor_tensor(
                        dst.rearrange("p (t c) -> p t c", c=P),
                        ps.rearrange("p (t c) -> p t c", c=P),
                        kvb.rearrange("p c -> p 1 c")[:, [0] * 8, :],
                        op=mybir.AluOpType.add)
                else:
                    nc.any.tensor_copy(dst, ps[:])

            # ---- q pass (d-ordered) + qk/AR per dq-half (mu) ----
            # Q2[p=hw, eta, d, c=(n,kd)]: drain lands in 512-contiguous runs
            # and the qk stationary [(dqh,n)-slice] is a stride-64 run.
            Q2d = Q2.rearrange("p (e d c) -> p e d c", e=2, d=D)
            # qk stationary view: [p, eta, mu, j=(dqh*8+n), kd]
            Q2v = Q2.rearrange("p (e m j k) -> p e m j k", e=2, m=2, k=KD)

            def qconv(d):
                ps = psum.tile([P, 1024], f32, tag="ps", name=f"psq{d}")
                for eta in range(2):
                    j = d * 2 + eta
                    for ci in range(2):
                        nc.tensor.matmul(ps[:, eta * 512:(eta + 1) * 512],
                                         x_sb[:, ci, j * P:(j + 1) * P],
                                         wq[:, ci, :],
                                         start=(ci == 0), stop=(ci == 1))
                if has_qb:
                    nc.any.tensor_tensor(
                        Q2d[:, :, d, :],
                        ps.rearrange("p (e f) -> p e f", e=2),
                        qb.rearrange("p f -> p 1 f")[:, [0, 0], :],
                        op=mybir.AluOpType.add)
                else:
                    nc.any.tensor_copy(Q2d[:, :, d, :],
                                       ps.rearrange("p (e f) -> p e f", e=2))

            def qk_ar(mu):
                psL = psum.tile([P, 32], f32, tag="ps", name=f"psL{mu}")
                for idx, (eta, kd) in enumerate((e, k) for e in range(2)
                                                for k in range(KD)):
                    nc.tensor.matmul(
                        psL[:],
                        Q2v[:, eta, mu, :, kd],
                        Kv[:, eta, kd, :],
                        start=(idx == 0), stop=(idx == 127))
                nc.any.tensor_copy(l2[:, mu, :], psL[:])
                nc.sync.dma_start(arin[mu][:], l2[:, mu, :])
                if sim_mode:
                    nc.sync.dma_start(arout[mu][:], arin[mu][:])
                else:
                    nc.gpsimd.collective_compute(
                        "AllReduce", mybir.AluOpType.add,
                        replica_groups=[[0, 1, 2, 3], [4, 5, 6, 7]],
                        ins=[arin[mu].opt()], outs=[arout[mu].opt()])

            # kv groups interleaved with d<16 q-convs (chunk g feeds both);
            # kv must fully finish before qk0, so d>=16 q-convs come after.
            for g in range(8):
                kvgroup(g)
                if g < 4:
                    for d in range(4 * g, 4 * g + 4):
                        qconv(d)
            # v re-layout on the otherwise-idle Pool engine (runs under qk/AR)
            Vsrc = kvsb.rearrange("p (d e ck vi v0) -> p ck e v0 vi d",
                                  d=D, e=2, ck=2, vi=4, v0=16)
            nc.gpsimd.tensor_copy(
                vst.rearrange("p (e v0 vi d) -> p e v0 vi d",
                              e=2, v0=16, vi=4),
                Vsrc[:, 1, :, :, :, :])
            qk_ar(0)
            for d in range(16, 32):
                qconv(d)
            qk_ar(1)
            # l3 loads emitted after both arin DMAs: the SP queue is
            # in-order, and l3[0] blocks on AR0 completion.
            for mu in range(2):
                nc.sync.dma_start(l3[:, mu, :], arout[mu][:])

            # ---- v transpose -> vq quads [vi*32+dk, (eta, v0, hw')] ----
            # hw' = w8*16 + hl*4 + wh (host-permuted token order), so the
            # av stationary col j2 = v0*8 + w8 is a stride-16 run.
            vstv = vst.rearrange("p (e v0 f) -> p e v0 f", e=2, v0=16)
            vqv = vq.rearrange("p (e v0 hw) -> p e v0 hw", e=2, v0=16)
            for eta in range(2):
                for g8 in range(2):
                    ps = psum.tile([P, 1024], bf16, tag="ps",
                                   name=f"psv{eta}_{g8}")
                    for l in range(8):
                        v0 = g8 * 8 + l
                        nc.tensor.transpose(ps[:, l * P:(l + 1) * P],
                                            vstv[:, eta, v0, :], idt[:])
                    nc.any.tensor_copy(
                        vqv[:, eta, g8 * 8:(g8 + 1) * 8, :],
                        ps.rearrange("p (l hw) -> p l hw", l=8))

            # oo[p=j2(v0,w8), c=vi, mu, w', nd] reuses Q2's slot
            oo = bigpool.tile([P, 4 * 2 * 32 * P], bf16, tag="big", name="oo")
            Oov = oo.rearrange("p (c m w nd) -> p c m w nd",
                               c=4, m=2, w=32)
            # av stationary view: [p=(vi,dk), eta, x=(hl,wh), j2]  (stride 16)
            Vat = vq.rearrange("p (e j2 x) -> p e x j2", e=2, j2=P, x=16)

            def softmax(mu):
                sl = l3[:, mu, :]
                mx = red[:, mu * 4 + 0: mu * 4 + 1]
                mxn = red[:, mu * 4 + 1: mu * 4 + 2]
                sm = red[:, mu * 4 + 2: mu * 4 + 3]
                rs = red[:, mu * 4 + 3: mu * 4 + 4]
                nc.vector.reduce_max(mx, sl, axis=AX.X, op=mybir.AluOpType.max)
                nc.scalar.mul(mxn, mx, -SCALE)
                nc.scalar.activation(ex[:, mu, :], sl, AF.Exp,
                                     bias=mxn, scale=SCALE, accum_out=sm)
                nc.vector.reciprocal(rs, sm)
                for vi in range(4):
                    nc.any.tensor_scalar_mul(
                        attn4[:, mu, vi * 32:(vi + 1) * 32], ex[:, mu, :], rs)

            def attn_t(mu):
                # attn transpose -> attnT4 [(vi,dk), (dqh,n)]
                pst = psum.tile([P, P], bf16, tag="ps", name=f"psat{mu}")
                nc.tensor.transpose(pst[:], attn4[:, mu, :], idt[:])
                nc.any.tensor_copy(attnT4[:, mu, :], pst[:])

            def av(mu):
                # av on 4 PE row-tiles (32x128), tile vi = vd quad
                for gg in range(4):      # 8 w' per psum tile
                    pls = [psum.tile([P, 1024], f32, tag="ps",
                                     name=f"psav{mu}_{gg}_{vi}")
                           for vi in range(4)]
                    for wl in range(8):
                        wp_ = gg * 8 + wl
                        e_w, x_w = wp_ // 16, wp_ % 16
                        for vi in range(4):
                            nc.tensor.matmul(
                                pls[vi][:, wl * P:(wl + 1) * P],
                                Vat[vi * 32:(vi + 1) * 32, e_w, x_w, :],
                                attnT4[vi * 32:(vi + 1) * 32, mu, :],
                                start=True, stop=True,
                                tile_position=(vi * 32, 0))
                    for vi in range(4):
                        nc.any.tensor_copy(
                            Oov[:, vi, mu, gg * 8:(gg + 1) * 8, :],
                            pls[vi].rearrange("p (w nd) -> p w nd", w=8))

            outv = out_t.rearrange("(ct p) t -> p ct t", p=P)

            def proj(mu):
                # proj (this mu's tokens) + layer_scale -> out
                for gg in range(4):      # 1024 tokens per gg
                    for ct in range(2):
                        ps = psum.tile([P, 1024], f32, tag="ps",
                                       name=f"psp{mu}_{gg}_{ct}")
                        for half in range(2):
                            for vi in range(4):
                                nc.tensor.matmul(
                                    ps[:, half * 512:(half + 1) * 512],
                                    wps[:, vi, ct * P:(ct + 1) * P],
                                    oo[:, vi * 8192 + mu * 4096 + gg * 1024
                                       + half * 512:
                                       vi * 8192 + mu * 4096 + gg * 1024
                                       + half * 512 + 512],
                                    start=(vi == 0), stop=(vi == 3))
                        stg = stpool.tile([P, 1024], f32, tag="stg",
                                          name=f"stg{mu}_{gg}_{ct}")
                        odd = (gg * 2 + ct) % 2
                        if has_pb:
                            nc.any.tensor_scalar_add(stg[:], ps[:],
                                                     pb[:, ct, :])
                        elif odd:
                            nc.scalar.copy(stg[:], ps[:])
                        else:
                            nc.vector.tensor_copy(stg[:], ps[:])
                        nc.sync.dma_start(
                            outv[:, ct,
                                 mu * 4096 + gg * 1024:mu * 4096 + (gg + 1) * 1024],
                            stg[:])

            # mu1's softmax is emitted before proj0 so its Scalar/Vector ops
            # don't queue behind proj0's PSUM drains; its PE transpose comes
            # after proj0 so a late AR1 can't stall proj0's matmuls.
            softmax(0)
            attn_t(0)
            av(0)
            softmax(1)
            proj(0)
            attn_t(1)
            av(1)
            proj(1)

    nc.finalize()
    return nc


def _get_nc(has_qb, has_kvb, has_pb):
    key = (has_qb, has_kvb, has_pb)
    if key not in _CACHE:
        _CACHE[key] = _build(*key)
    return _CACHE[key]


def kernel(x, q_w, q_b, kv_w, kv_b, proj_w, proj_b, layer_scale):
    from concourse.bass_utils import run_bass_kernel_spmd
    import os

    x = np.asarray(x, dtype=np.float32)
    q_w = np.asarray(q_w, dtype=np.float32)
    q_b = np.asarray(q_b, dtype=np.float32)
    kv_w = np.asarray(kv_w, dtype=np.float32)
    kv_b = np.asarray(kv_b, dtype=np.float32)
    proj_w = np.asarray(proj_w, dtype=np.float32)
    proj_b = np.asarray(proj_b, dtype=np.float32)
    layer_scale = np.asarray(layer_scale, dtype=np.float32)

    has_qb = bool(np.any(q_b != 0))
    has_kvb = bool(np.any(kv_b != 0))
    has_pb = bool(np.any(proj_b != 0))
    nc = _get_nc(has_qb, has_kvb, has_pb)

    bf = ml_dtypes.bfloat16
    ls_c = layer_scale.reshape(C)                          # [C] f32
    wq = np.ascontiguousarray(q_w.T).astype(bf)            # [C, 512]
    wkv = np.ascontiguousarray(kv_w.T).astype(bf)          # [C, 128]
    # fold layer_scale into proj weights; row j2 = v0*8 + w8 of block vi
    # holds c2 = w8*64 + 16*vi + v0
    wp_full = (proj_w * ls_c[:, None]).T                   # [512 = c2, C]
    j2 = np.arange(P)
    rows = (j2 % 8)[:, None] * 64 + 16 * np.arange(4)[None, :] \
        + (j2 // 8)[:, None]                               # [128, 4vi]
    wps = np.ascontiguousarray(
        wp_full[rows].reshape(P, 4 * C)).astype(bf)        # [128, 4*C]
    idt = np.eye(P, dtype=bf)

    shared = {"wq": wq, "wkv": wkv, "wp": wps, "idt": idt}
    if has_qb:
        shared["qb"] = np.broadcast_to(q_b.astype(bf), (P, NH * KD)).copy()
    if has_kvb:
        shared["kvb"] = np.broadcast_to(kv_b.astype(bf), (P, KD + VD)).copy()
    if has_pb:
        shared["pb"] = (proj_b * layer_scale.reshape(-1)).reshape(C, 1) \
            .astype(np.float32)

    in_maps = []
    for c in range(NCORES):
        b, hg = c // 4, c % 4
        # token order: (d, eta=hl//4, w8=w%8, hl=h%4, wh=w//8)
        xs = x[b, :, :, hg * HS:(hg + 1) * HS, :] \
            .reshape(C, D, 2, 4, 4, 8).transpose(0, 1, 2, 5, 3, 4)
        xc = np.ascontiguousarray(xs.reshape(C, T)).astype(bf)
        in_maps.append({"x": xc, **shared})

    trace = bool(int(os.environ.get("KERNEL_TRACE", "0")))
    res = run_bass_kernel_spmd(nc, in_maps, core_ids=list(range(NCORES)),
                               trace=trace)
    kernel.last_results = res

    out = np.empty((B, C, D, H, W), dtype=np.float32)
    for c in range(NCORES):
        b, hg = c // 4, c % 4
        # res T enum = (mu, w', dqh, n): d = mu*16+dqh, h' = n*4+hg
        r = res.results[c]["out"].reshape(C, 2, 32, 16, NH)
        r = r.transpose(0, 1, 3, 4, 2).reshape(C, D, NH, 32)
        out[b, :, :, hg::4, :] = r
    return out
